# revision 36
# baseline (speedup 1.0000x reference)
"""Adjacency-aware multi-head attention on 8 trn2 NeuronCores.

Math (per b, head k):
  Q = h[b] @ Wq[:, k] + bq[k]           [N, D]
  S[i, j] = (Q_i . K_j) / sqrt(D)
  P[j, i] = exp(S[i, j]) / sum_j exp(S[i, j])      (softmax over keys j)
  out[i, d] = sum_j P[j, i] * A[b, j, i] * V[j, d]

Sharding: 16 (b, head) pairs over 8 cores, 2 heads of the SAME b per core so
the A[b] stream is shared by both heads.

Device dataflow ([j, i] "transposed" layout so A needs no transpose):
  S^T[j-tile, i-chunk] on PE: K^T tile stationary (bf16), Q^T moving (bf16).
    K^T is packed so j-tile t lives on partition strip 32*(t%4) and Q^T is
    replicated to all 4 strips -> 4 j-tiles run CONCURRENTLY on the PE's
    four 32-row groups.
  exp on ACT: PSUM -> SBUF bf16 (groups of 3 PSUM banks per op)
  EA = E * A on DVE (bf16 tensor_tensor, 2x mode; A broadcast over heads)
  Phase 2 (lagged one chunk behind S so the in-order PE queue never stalls),
  4 concurrent col-group accumulation streams into one PSUM tile:
    cols  0-31: outT_h0 += V_h0[j]^T @ EA_h0     (M=32)
    cols 32-63: outT_h1 += V_h1[j]^T @ EA_h1     (M=32)
    col  64: denom_h0 += ones^T @ E_h0           (M=1)
    col  96: denom_h1 += ones^T @ E_h1           (M=1)
Device returns [128, N]: rows 0-31 outT_h0, 32-63 outT_h1, rows 64/96 the
softmax denominators.  Host does out = (outT / denom)^T plus the gather.
"""

import math
import os

import numpy as np
import ml_dtypes

B, N, IN_DIM = 2, 2048, 256
HEADS, D = 8, 32
NCORES = 8
HPC = 2              # heads per core
NJ = N // 128        # 16 j-tiles
NCH = 4              # i-chunks
CH = N // NCH        # 512
CORES_PER_B = NCORES // B

LAST_RESULTS = None  # BassKernelResults of the most recent kernel() call


def _build_bass():
    import concourse.bass as bass
    import concourse.mybir as mybir
    import concourse.tile as tile
    from concourse import bacc

    f32 = mybir.dt.float32
    bf16 = mybir.dt.bfloat16
    AF = mybir.ActivationFunctionType

    nc = bacc.Bacc("TRN2", target_bir_lowering=False, debug=False,
                   num_devices=NCORES)

    hT = nc.dram_tensor("hT", [IN_DIM, N], bf16, kind="ExternalInput").ap()
    Ab = nc.dram_tensor("Ab", [N, N], bf16, kind="ExternalInput").ap()
    # q/k/v weights packed in ONE tensor (one DMA = one serialized
    # descriptor-gen slot instead of three); biases likewise.
    wqkv = nc.dram_tensor("wqkv", [IN_DIM, 3 * HPC * D], bf16,
                          kind="ExternalInput").ap()
    bcat = nc.dram_tensor("bcat", [128, 5], f32, kind="ExternalInput").ap()
    o = nc.dram_tensor("o", [128, N], f32, kind="ExternalOutput").ap()

    SC = 1.0 / math.sqrt(D)
    WQ, WK, WV = 0, HPC * D, 2 * HPC * D
    # Schraudolph exp in bf16 bit space: bf16bits(e^s) ~ int16(s*SALPHA+SBETA)
    SALPHA = 128.0 / math.log(2.0)
    SBETA = 127.0 * 128.0 - 5.5

    with (
        tile.TileContext(nc) as tc,
        tc.tile_pool(name="const", bufs=1) as cpool,
        tc.tile_pool(name="ps", bufs=2, space="PSUM") as pspool,
        tc.tile_pool(name="pod", bufs=2, space="PSUM") as podpool,
        tc.tile_pool(name="apool", bufs=2) as apool,
        tc.tile_pool(name="epool", bufs=2) as epool,
        tc.tile_pool(name="eapool", bufs=2) as eapool,
        tc.tile_pool(name="opool", bufs=3) as opool,
    ):
        # ---- constants / inputs into SBUF (hT split by s-half so K-proj
        #      s=0 matmuls can start as soon as the first half lands)
        hT3 = hT.rearrange("(s p) n -> p s n", p=128)
        hT_sb = cpool.tile([128, 2, N], bf16, tag="hT")
        for s in range(2):
            nc.sync.dma_start(hT_sb[:, s, :], hT3[:, s, :])
        w_sb = cpool.tile([128, 2, 3 * HPC * D], bf16, tag="w")
        nc.sync.dma_start(w_sb, wqkv.rearrange("(s p) m -> p s m", p=128))
        bcat_sb = cpool.tile([128, 5], f32, tag="bcat")
        nc.sync.dma_start(bcat_sb, bcat)
        bq4_sb = bcat_sb[:, 0:2]
        bk4_sb = bcat_sb[:, 2:4]
        bvt_sb = bcat_sb[0:HPC * D, 4:5]
        ones_sb = cpool.tile([128, 1], bf16, tag="ones")
        nc.vector.memset(ones_sb, 1.0)
        sbeta_sb = cpool.tile([128, 1], f32, tag="sbeta")
        nc.vector.memset(sbeta_sb, SBETA)


        # Q^T replicated on all 4 partition strips; K^T packed so j-tile t
        # sits on strip 32*(t%4), column block t//4.  One tile per QUARTER
        # (not one per head) so a quarter written mid-stream never forms a
        # false WAR against earlier chunks' S-block reads (Tile tracks tile
        # deps coarsely).
        qt4 = [[cpool.tile([128, CH], bf16, tag=f"qt4{h}_{q}",
                           name=f"qt4{h}{q}") for q in range(4)]
               for h in range(HPC)]
        kt4 = [cpool.tile([128, NJ // 4, 128], bf16, tag=f"kt4{h}",
                          name=f"kt4{h}") for h in range(HPC)]
        Vt = cpool.tile([128, NJ, HPC * D], bf16, tag="vt")

        def bcast_free(ap_col, n):
            return bass.AP(tensor=ap_col.tensor, offset=ap_col.offset,
                           ap=[ap_col.ap[0], [0, n]])

        # ---- K projection straight into the packed kt4 layout: for strip r
        #      the moving operand picks j-tiles {r, r+4, r+8, r+12}
        def emit_kproj(h):
            ps = pspool.tile([128, 4 * 128], f32, tag="ps", name="kps")
            for r in range(4):
                for s in range(2):
                    base = hT_sb[:, s, r * 128:(r + 1) * 128]
                    rhs = bass.AP(
                        tensor=base.tensor, offset=base.offset,
                        ap=[base.ap[0], [4 * 128, 4], [1, 128]],
                    )
                    nc.tensor.matmul(
                        ps[32 * r:32 * r + 32, :],
                        lhsT=w_sb[:, s, WK + h * D:WK + (h + 1) * D],
                        rhs=rhs,
                        start=(s == 0), stop=(s == 1),
                        tile_position=(0, 32 * r),
                    )
            nc.vector.scalar_tensor_tensor(
                kt4[h].rearrange("p q jj -> p (q jj)"), ps, 1.0,
                bcast_free(bk4_sb[:, h:h + 1], 4 * 128),
                op0=mybir.AluOpType.mult, op1=mybir.AluOpType.add,
            )

        # V projection with Wv STATIONARY (2 weight loads total instead of a
        # 128x128 LDWEIGHTS per j-tile): V^T[d, j] streamed into PSUM, bias
        # added on DVE into Vb, then 16 xbar DMA transposes into the [j, d]
        # layout phase2 needs.
        Vb = cpool.tile([HPC * D, N], bf16, tag="vb")

        def emit_vproj():
            bvt_bcast = bcast_free(bvt_sb, CH)
            for qv in range(4):
                ps = podpool.tile([128, CH], f32, tag="od", name="vps")
                for s in range(2):
                    nc.tensor.matmul(
                        ps[0:HPC * D, :],
                        lhsT=w_sb[:, s, WV:WV + HPC * D],
                        rhs=hT_sb[:, s, qv * CH:(qv + 1) * CH],
                        start=(s == 0), stop=(s == 1),
                    )
                nc.vector.scalar_tensor_tensor(
                    Vb[:, qv * CH:(qv + 1) * CH], ps[0:HPC * D, :], 1.0,
                    bvt_bcast,
                    op0=mybir.AluOpType.mult, op1=mybir.AluOpType.add,
                )
            for t in range(NJ):
                nc.sync.dma_start_transpose(
                    Vt[:, t, :], Vb[:, t * 128:(t + 1) * 128])

        # ---- Q projection (+bias, scaled 1/sqrt(D)) per quarter.  The
        #      4-strip replication is done ON THE PE via column-position
        #      tiling (4 concurrent col-groups write partitions 32r), so no
        #      SBUF->SBUF replication DMAs exist at all.
        def emit_qproj(quarter):
            sl = slice(quarter * CH, (quarter + 1) * CH)
            ps = pspool.tile([128, HPC * CH], f32, tag="ps", name="qps")
            for h in range(HPC):
                for s in range(2):
                    for r in range(4):
                        nc.tensor.matmul(
                            ps[32 * r:32 * r + 32, h * CH:(h + 1) * CH],
                            lhsT=w_sb[:, s, WQ + h * D:WQ + (h + 1) * D],
                            rhs=hT_sb[:, s, sl],
                            start=(s == 0), stop=(s == 1),
                            tile_position=(0, 32 * r),
                        )
            for h in range(HPC):
                nc.vector.scalar_tensor_tensor(
                    qt4[h][quarter], ps[:, h * CH:(h + 1) * CH], SC,
                    bcast_free(bq4_sb[:, h:h + 1], CH),
                    op0=mybir.AluOpType.mult, op1=mybir.AluOpType.add,
                )

        # ---- main loop (phase2 lags one chunk so the in-order PE queue
        #      never stalls waiting on exp/A-mult of the same chunk)
        A3 = Ab.rearrange("(t p) i -> p t i", p=128)

        # e/ea column block for (head hh, j-tile t):
        def blk(hh, t):
            return (t // 4) * 8 + hh * 4 + (t % 4)

        def emit_phase2_tile(od, e_t, ea_t, t):
            first, last = (t == 0), (t == NJ - 1)
            sh0 = slice(blk(0, t) * CH, (blk(0, t) + 1) * CH)
            sh1 = slice(blk(1, t) * CH, (blk(1, t) + 1) * CH)
            nc.tensor.matmul(
                od[0:D, :], lhsT=Vt[:, t, 0:D], rhs=ea_t[:, sh0],
                start=first, stop=last, tile_position=(0, 0),
            )
            nc.tensor.matmul(
                od[D:2 * D, :], lhsT=Vt[:, t, D:2 * D], rhs=ea_t[:, sh1],
                start=first, stop=last, tile_position=(0, 32),
            )
            nc.tensor.matmul(
                od[64:65, :], lhsT=ones_sb, rhs=e_t[:, sh0],
                start=first, stop=last, tile_position=(0, 64),
            )
            nc.tensor.matmul(
                od[96:97, :], lhsT=ones_sb, rhs=e_t[:, sh1],
                start=first, stop=last, tile_position=(0, 96),
            )

        def emit_phase2(e_t, ea_t, ch):
            od = podpool.tile([128, CH], f32, tag="od")
            for t in range(NJ):
                emit_phase2_tile(od, e_t, ea_t, t)
            o_sb = opool.tile([128, CH], f32, tag="o")
            nc.vector.tensor_copy(o_sb, od)
            nc.sync.dma_start(o[:, ch * CH:(ch + 1) * CH], o_sb)

        GRP = 3              # S psum banks per exp op
        nblocks = NJ * HPC

        # S matmuls: block b = q*8 + hh*4 + r covers j-tile t = 4q + r;
        # runs of 4 same-head blocks occupy the 4 PE row-groups and run
        # concurrently.  exp drains GRP psum banks per op.
        def emit_sblocks(ch, e_t, b_lo, b_hi):
            ps = None
            gsz = 0
            for b in range(b_lo, b_hi):
                q, hh, r = b // 8, (b // 4) % 2, b % 4
                g = (b - b_lo) % GRP
                if g == 0:
                    gsz = min(GRP, b_hi - b)
                    ps = pspool.tile([128, GRP * CH], f32, tag="ps",
                                     name="sps")
                nc.tensor.matmul(
                    ps[:, g * CH:(g + 1) * CH],
                    lhsT=kt4[hh][32 * r:32 * r + 32, q, :],
                    rhs=qt4[hh][ch][32 * r:32 * r + 32, :],
                    start=True, stop=True,
                    tile_position=(32 * r, 0),
                )
                if g == gsz - 1:
                    b0 = b - g
                    # Split each exp group: ACT (the saturated engine) takes
                    # the first part, the DVE builds the bf16 bits of e^s
                    # for the tail via one fused mult+add into int16
                    # (Schraudolph).  No ACT op is removed, so the exp-paced
                    # PSUM slot pipeline keeps its cadence.
                    L = gsz * CH
                    XG = L // 8
                    nc.scalar.activation(
                        e_t[:, b0 * CH:b0 * CH + L - XG], ps[:, :L - XG],
                        AF.Exp)
                    nc.vector.scalar_tensor_tensor(
                        e_t[:, b0 * CH + L - XG:(b + 1) * CH].bitcast(
                            mybir.dt.int16),
                        ps[:, L - XG:L], SALPHA,
                        bcast_free(sbeta_sb, XG),
                        op0=mybir.AluOpType.mult,
                        op1=mybir.AluOpType.add,
                    )

        def emit_ea(e_t, ea_t, a_t, t, ntile=1):
            # one DVE op covering heads x ntile consecutive j-tiles (tiles
            # 4q..4q+3 are contiguous per head in the e/ea block layout)
            b0 = blk(0, t)
            base = e_t[:, b0 * CH:(b0 + 1) * CH]
            e_pair = bass.AP(tensor=base.tensor, offset=base.offset,
                             ap=[base.ap[0], [4 * CH, 2], [1, ntile * CH]])
            base2 = ea_t[:, b0 * CH:(b0 + 1) * CH]
            ea_pair = bass.AP(tensor=base2.tensor, offset=base2.offset,
                              ap=[base2.ap[0], [4 * CH, 2], [1, ntile * CH]])
            a_sl = a_t[:, t * CH:(t + 1) * CH]
            a_bcast = bass.AP(tensor=a_sl.tensor, offset=a_sl.offset,
                              ap=[a_sl.ap[0], [0, HPC], [1, ntile * CH]])
            nc.vector.tensor_mul(ea_pair, e_pair, a_bcast)

        def new_chunk(ch):
            a_t = apool.tile([128, NJ * CH], bf16, tag="a")
            # sim-time floor keeps later A loads from being queue-ordered
            # ahead of latency-critical small DMAs (e.g. Vt transposes)
            with tc.tile_wait_until(0.016 * ch, enable=ch > 0):
                nc.sync.dma_start(
                    a_t.rearrange("p (t i) -> p t i", i=CH),
                    A3[:, :, ch * CH:(ch + 1) * CH],
                )
            e_t = epool.tile([128, nblocks * CH], bf16, tag="e")
            ea_t = eapool.tile([128, nblocks * CH], bf16, tag="ea")
            return a_t, e_t, ea_t

        # ---- projections all up front (any quarter of Q written mid-stream
        #      forms a false WAR against earlier S-block reads).  Ordered so
        #      the first exp group's deps (K h0, Q quarter 0) come first.
        emit_kproj(0)
        emit_qproj(0)
        emit_kproj(1)
        for quarter in range(1, 4):
            emit_qproj(quarter)
        emit_vproj()

        # ---- main loop, baseline structure: per chunk emit all S blocks +
        #      exps, then the EA multiplies (4 j-tiles fused), then the
        #      LAGGED phase2 of the previous chunk as one block.
        pending = None
        for ch in range(NCH - 1):
            a_t, e_t, ea_t = new_chunk(ch)
            emit_sblocks(ch, e_t, 0, nblocks)
            for q in range(4):
                emit_ea(e_t, ea_t, a_t, 4 * q, ntile=4)
            if pending is not None:
                emit_phase2(*pending)
            pending = (e_t, ea_t, ch)

        # ---- last chunk: phase2[2] right after the S blocks, then this
        #      chunk's own phase2 per j-tile behind each EA multiply so the
        #      tail after the final exp is ~one tile instead of a chunk.
        ch = NCH - 1
        a_t, e_t, ea_t = new_chunk(ch)
        emit_sblocks(ch, e_t, 0, nblocks)
        emit_phase2(*pending)
        od = podpool.tile([128, CH], f32, tag="od")
        for t in range(NJ):
            emit_ea(e_t, ea_t, a_t, t)
        for t in range(NJ):
            emit_phase2_tile(od, e_t, ea_t, t)
        o_sb = opool.tile([128, CH], f32, tag="o")
        nc.vector.tensor_copy(o_sb, od)
        nc.sync.dma_start(o[:, ch * CH:(ch + 1) * CH], o_sb)

    nc.finalize()
    return nc


def kernel(h, A, Wq, bq, Wk, bk, Wv, bv):
    global LAST_RESULTS
    from concourse.bass_utils import run_bass_kernel_spmd

    h = np.asarray(h, np.float32)
    A = np.asarray(A, np.float32)
    Wq = np.asarray(Wq, np.float32)
    Wk = np.asarray(Wk, np.float32)
    Wv = np.asarray(Wv, np.float32)
    bq = np.asarray(bq, np.float32)
    bk = np.asarray(bk, np.float32)
    bv = np.asarray(bv, np.float32)

    hT = np.ascontiguousarray(h.transpose(0, 2, 1)).astype(ml_dtypes.bfloat16)
    Ab = np.ascontiguousarray(A.astype(ml_dtypes.bfloat16))  # [B, N, N]
    sc = np.float32(1.0 / math.sqrt(D))

    in_maps = []
    for c in range(NCORES):
        b = c // CORES_PER_B
        h0 = HPC * (c % CORES_PER_B)
        sl = slice(h0 * D, (h0 + HPC) * D)
        bk2 = bk[sl].reshape(HPC, D)                    # [head, d]
        bq2 = (bq[sl] * sc).reshape(HPC, D)
        bcat = np.zeros((128, 5), np.float32)
        for hh in range(HPC):
            bcat[:, 0 + hh] = np.tile(bq2[hh], 4)       # strip-replicated
            bcat[:, 2 + hh] = np.tile(bk2[hh], 4)
        bcat[0:HPC * D, 4] = bv[sl]
        wqkv = np.concatenate([Wq[:, sl], Wk[:, sl], Wv[:, sl]], axis=1)
        in_maps.append({
            "hT": hT[b],
            "Ab": Ab[b],
            "wqkv": np.ascontiguousarray(wqkv).astype(ml_dtypes.bfloat16),
            "bcat": bcat,
        })

    nc = _build_bass()
    res = run_bass_kernel_spmd(
        nc, in_maps, core_ids=list(range(NCORES)),
        trace=os.environ.get("BASS_TRACE", "0") == "1",
    )
    LAST_RESULTS = res

    out = np.empty((B, HEADS, N, D), np.float32)
    for c in range(NCORES):
        b = c // CORES_PER_B
        h0 = HPC * (c % CORES_PER_B)
        oo = res.results[c]["o"]                  # [128, N] f32
        for hh in range(HPC):
            num = oo[hh * D:(hh + 1) * D, :]      # [32, N] unnormalized out^T
            den = oo[64 + 32 * hh, :]             # [N]
            out[b, h0 + hh] = (num / den[None, :]).T
    return out



# revision 38
# speedup vs baseline: 1.0081x; 1.0081x over previous
"""Adjacency-aware multi-head attention on 8 trn2 NeuronCores.

Math (per b, head k):
  Q = h[b] @ Wq[:, k] + bq[k]           [N, D]
  S[i, j] = (Q_i . K_j) / sqrt(D)
  P[j, i] = exp(S[i, j]) / sum_j exp(S[i, j])      (softmax over keys j)
  out[i, d] = sum_j P[j, i] * A[b, j, i] * V[j, d]

Sharding: 16 (b, head) pairs over 8 cores, 2 heads of the SAME b per core so
the A[b] stream is shared by both heads.

Device dataflow ([j, i] "transposed" layout so A needs no transpose):
  S^T[j-tile, i-chunk] on PE: K^T tile stationary (bf16), Q^T moving (bf16).
    K^T is packed so j-tile t lives on partition strip 32*(t%4) and Q^T is
    replicated to all 4 strips -> 4 j-tiles run CONCURRENTLY on the PE's
    four 32-row groups.
  exp on ACT: PSUM -> SBUF bf16 (groups of 3 PSUM banks per op)
  EA = E * A on DVE (bf16 tensor_tensor, 2x mode; A broadcast over heads)
  Phase 2 (lagged one chunk behind S so the in-order PE queue never stalls),
  4 concurrent col-group accumulation streams into one PSUM tile:
    cols  0-31: outT_h0 += V_h0[j]^T @ EA_h0     (M=32)
    cols 32-63: outT_h1 += V_h1[j]^T @ EA_h1     (M=32)
    col  64: denom_h0 += ones^T @ E_h0           (M=1)
    col  96: denom_h1 += ones^T @ E_h1           (M=1)
Device returns [128, N]: rows 0-31 outT_h0, 32-63 outT_h1, rows 64/96 the
softmax denominators.  Host does out = (outT / denom)^T plus the gather.
"""

import math
import os

import numpy as np
import ml_dtypes

B, N, IN_DIM = 2, 2048, 256
HEADS, D = 8, 32
NCORES = 8
HPC = 2              # heads per core
NJ = N // 128        # 16 j-tiles
NCH = 4              # i-chunks
CH = N // NCH        # 512
CORES_PER_B = NCORES // B

LAST_RESULTS = None  # BassKernelResults of the most recent kernel() call


def _build_bass():
    import concourse.bass as bass
    import concourse.mybir as mybir
    import concourse.tile as tile
    from concourse import bacc

    f32 = mybir.dt.float32
    bf16 = mybir.dt.bfloat16
    AF = mybir.ActivationFunctionType

    nc = bacc.Bacc("TRN2", target_bir_lowering=False, debug=False,
                   num_devices=NCORES)

    hT = nc.dram_tensor("hT", [IN_DIM, N], bf16, kind="ExternalInput").ap()
    Ab = nc.dram_tensor("Ab", [N, N], bf16, kind="ExternalInput").ap()
    # q/k/v weights packed in ONE tensor (one DMA = one serialized
    # descriptor-gen slot instead of three); biases likewise.
    wqkv = nc.dram_tensor("wqkv", [IN_DIM, 3 * HPC * D], bf16,
                          kind="ExternalInput").ap()
    bcat = nc.dram_tensor("bcat", [128, 5], f32, kind="ExternalInput").ap()
    o = nc.dram_tensor("o", [128, N], f32, kind="ExternalOutput").ap()

    SC = 1.0 / math.sqrt(D)
    WQ, WK, WV = 0, HPC * D, 2 * HPC * D
    # Schraudolph exp in bf16 bit space: bf16bits(e^s) ~ int16(s*SALPHA+SBETA)
    SALPHA = 128.0 / math.log(2.0)
    SBETA = 127.0 * 128.0 - 5.5

    with (
        tile.TileContext(nc) as tc,
        tc.tile_pool(name="const", bufs=1) as cpool,
        tc.tile_pool(name="ps", bufs=2, space="PSUM") as pspool,
        tc.tile_pool(name="pod", bufs=2, space="PSUM") as podpool,
        tc.tile_pool(name="apool", bufs=2) as apool,
        tc.tile_pool(name="epool", bufs=2) as epool,
        tc.tile_pool(name="eapool", bufs=2) as eapool,
        tc.tile_pool(name="opool", bufs=3) as opool,
    ):
        # ---- constants / inputs into SBUF (hT split by s-half so K-proj
        #      s=0 matmuls can start as soon as the first half lands)
        hT3 = hT.rearrange("(s p) n -> p s n", p=128)
        hT_sb = cpool.tile([128, 2, N], bf16, tag="hT")
        for s in range(2):
            nc.sync.dma_start(hT_sb[:, s, :], hT3[:, s, :])
        w_sb = cpool.tile([128, 2, 3 * HPC * D], bf16, tag="w")
        nc.sync.dma_start(w_sb, wqkv.rearrange("(s p) m -> p s m", p=128))
        bcat_sb = cpool.tile([128, 5], f32, tag="bcat")
        nc.sync.dma_start(bcat_sb, bcat)
        bq4_sb = bcat_sb[:, 0:2]
        bk4_sb = bcat_sb[:, 2:4]
        bvt_sb = bcat_sb[0:HPC * D, 4:5]
        ones_sb = cpool.tile([128, 1], bf16, tag="ones")
        nc.vector.memset(ones_sb, 1.0)
        sbeta_sb = cpool.tile([128, 1], f32, tag="sbeta")
        nc.vector.memset(sbeta_sb, SBETA)


        # Q^T replicated on all 4 partition strips; K^T packed so j-tile t
        # sits on strip 32*(t%4), column block t//4.  One tile per QUARTER
        # (not one per head) so a quarter written mid-stream never forms a
        # false WAR against earlier chunks' S-block reads (Tile tracks tile
        # deps coarsely).
        qt4 = [[cpool.tile([128, CH], bf16, tag=f"qt4{h}_{q}",
                           name=f"qt4{h}{q}") for q in range(4)]
               for h in range(HPC)]
        kt4 = [cpool.tile([128, NJ // 4, 128], bf16, tag=f"kt4{h}",
                          name=f"kt4{h}") for h in range(HPC)]
        Vt = cpool.tile([128, NJ, HPC * D], bf16, tag="vt")

        def bcast_free(ap_col, n):
            return bass.AP(tensor=ap_col.tensor, offset=ap_col.offset,
                           ap=[ap_col.ap[0], [0, n]])

        # ---- K projection straight into the packed kt4 layout: for strip r
        #      the moving operand picks j-tiles {r, r+4, r+8, r+12}
        def emit_kproj(h):
            ps = pspool.tile([128, 4 * 128], f32, tag="ps", name="kps")
            for r in range(4):
                for s in range(2):
                    base = hT_sb[:, s, r * 128:(r + 1) * 128]
                    rhs = bass.AP(
                        tensor=base.tensor, offset=base.offset,
                        ap=[base.ap[0], [4 * 128, 4], [1, 128]],
                    )
                    nc.tensor.matmul(
                        ps[32 * r:32 * r + 32, :],
                        lhsT=w_sb[:, s, WK + h * D:WK + (h + 1) * D],
                        rhs=rhs,
                        start=(s == 0), stop=(s == 1),
                        tile_position=(0, 32 * r),
                    )
            nc.vector.scalar_tensor_tensor(
                kt4[h].rearrange("p q jj -> p (q jj)"), ps, 1.0,
                bcast_free(bk4_sb[:, h:h + 1], 4 * 128),
                op0=mybir.AluOpType.mult, op1=mybir.AluOpType.add,
            )

        # V projection with Wv STATIONARY (2 weight loads total instead of a
        # 128x128 LDWEIGHTS per j-tile): V^T[d, j] streamed into PSUM, bias
        # added on DVE into Vb, then 16 xbar DMA transposes into the [j, d]
        # layout phase2 needs.
        Vb = cpool.tile([HPC * D, N], bf16, tag="vb")

        def emit_vproj():
            bvt_bcast = bcast_free(bvt_sb, CH)
            for qv in range(4):
                ps = podpool.tile([128, CH], f32, tag="od", name="vps")
                for s in range(2):
                    nc.tensor.matmul(
                        ps[0:HPC * D, :],
                        lhsT=w_sb[:, s, WV:WV + HPC * D],
                        rhs=hT_sb[:, s, qv * CH:(qv + 1) * CH],
                        start=(s == 0), stop=(s == 1),
                    )
                nc.vector.scalar_tensor_tensor(
                    Vb[:, qv * CH:(qv + 1) * CH], ps[0:HPC * D, :], 1.0,
                    bvt_bcast,
                    op0=mybir.AluOpType.mult, op1=mybir.AluOpType.add,
                )
            for t in range(NJ):
                nc.sync.dma_start_transpose(
                    Vt[:, t, :], Vb[:, t * 128:(t + 1) * 128])

        # ---- Q projection (+bias, scaled 1/sqrt(D)) per quarter.  The
        #      4-strip replication is done ON THE PE via column-position
        #      tiling (4 concurrent col-groups write partitions 32r), so no
        #      SBUF->SBUF replication DMAs exist at all.
        def emit_qproj(quarter):
            sl = slice(quarter * CH, (quarter + 1) * CH)
            ps = pspool.tile([128, HPC * CH], f32, tag="ps", name="qps")
            for h in range(HPC):
                for s in range(2):
                    for r in range(4):
                        nc.tensor.matmul(
                            ps[32 * r:32 * r + 32, h * CH:(h + 1) * CH],
                            lhsT=w_sb[:, s, WQ + h * D:WQ + (h + 1) * D],
                            rhs=hT_sb[:, s, sl],
                            start=(s == 0), stop=(s == 1),
                            tile_position=(0, 32 * r),
                        )
            for h in range(HPC):
                nc.vector.scalar_tensor_tensor(
                    qt4[h][quarter], ps[:, h * CH:(h + 1) * CH], SC,
                    bcast_free(bq4_sb[:, h:h + 1], CH),
                    op0=mybir.AluOpType.mult, op1=mybir.AluOpType.add,
                )

        # ---- main loop (phase2 lags one chunk so the in-order PE queue
        #      never stalls waiting on exp/A-mult of the same chunk)
        A3 = Ab.rearrange("(t p) i -> p t i", p=128)

        # e/ea column block for (head hh, j-tile t):
        def blk(hh, t):
            return (t // 4) * 8 + hh * 4 + (t % 4)

        def emit_phase2_tile(od, e_t, ea_t, t):
            first, last = (t == 0), (t == NJ - 1)
            sh0 = slice(blk(0, t) * CH, (blk(0, t) + 1) * CH)
            sh1 = slice(blk(1, t) * CH, (blk(1, t) + 1) * CH)
            nc.tensor.matmul(
                od[0:D, :], lhsT=Vt[:, t, 0:D], rhs=ea_t[:, sh0],
                start=first, stop=last, tile_position=(0, 0),
            )
            nc.tensor.matmul(
                od[D:2 * D, :], lhsT=Vt[:, t, D:2 * D], rhs=ea_t[:, sh1],
                start=first, stop=last, tile_position=(0, 32),
            )
            nc.tensor.matmul(
                od[64:65, :], lhsT=ones_sb, rhs=e_t[:, sh0],
                start=first, stop=last, tile_position=(0, 64),
            )
            nc.tensor.matmul(
                od[96:97, :], lhsT=ones_sb, rhs=e_t[:, sh1],
                start=first, stop=last, tile_position=(0, 96),
            )

        def emit_phase2(e_t, ea_t, ch):
            od = podpool.tile([128, CH], f32, tag="od")
            for t in range(NJ):
                emit_phase2_tile(od, e_t, ea_t, t)
            o_sb = opool.tile([128, CH], f32, tag="o")
            nc.vector.tensor_copy(o_sb, od)
            nc.sync.dma_start(o[:, ch * CH:(ch + 1) * CH], o_sb)

        GRP = 3              # S psum banks per exp op
        nblocks = NJ * HPC

        # S matmuls: block b = q*8 + hh*4 + r covers j-tile t = 4q + r;
        # runs of 4 same-head blocks occupy the 4 PE row-groups and run
        # concurrently.  exp drains GRP psum banks per op.
        def emit_sblocks(ch, e_t, b_lo, b_hi):
            ps = None
            gsz = 0
            for b in range(b_lo, b_hi):
                q, hh, r = b // 8, (b // 4) % 2, b % 4
                g = (b - b_lo) % GRP
                if g == 0:
                    gsz = min(GRP, b_hi - b)
                    ps = pspool.tile([128, GRP * CH], f32, tag="ps",
                                     name="sps")
                nc.tensor.matmul(
                    ps[:, g * CH:(g + 1) * CH],
                    lhsT=kt4[hh][32 * r:32 * r + 32, q, :],
                    rhs=qt4[hh][ch][32 * r:32 * r + 32, :],
                    start=True, stop=True,
                    tile_position=(32 * r, 0),
                )
                if g == gsz - 1:
                    b0 = b - g
                    # Split each exp group: ACT (the saturated engine) takes
                    # the first part, the DVE builds the bf16 bits of e^s
                    # for the tail via one fused mult+add into int16
                    # (Schraudolph).  No ACT op is removed, so the exp-paced
                    # PSUM slot pipeline keeps its cadence.
                    L = gsz * CH
                    XG = 128
                    nc.scalar.activation(
                        e_t[:, b0 * CH:b0 * CH + L - XG], ps[:, :L - XG],
                        AF.Exp)
                    nc.vector.scalar_tensor_tensor(
                        e_t[:, b0 * CH + L - XG:(b + 1) * CH].bitcast(
                            mybir.dt.int16),
                        ps[:, L - XG:L], SALPHA,
                        bcast_free(sbeta_sb, XG),
                        op0=mybir.AluOpType.mult,
                        op1=mybir.AluOpType.add,
                    )

        def emit_ea(e_t, ea_t, a_t, t, ntile=1):
            # one DVE op covering heads x ntile consecutive j-tiles (tiles
            # 4q..4q+3 are contiguous per head in the e/ea block layout)
            b0 = blk(0, t)
            base = e_t[:, b0 * CH:(b0 + 1) * CH]
            e_pair = bass.AP(tensor=base.tensor, offset=base.offset,
                             ap=[base.ap[0], [4 * CH, 2], [1, ntile * CH]])
            base2 = ea_t[:, b0 * CH:(b0 + 1) * CH]
            ea_pair = bass.AP(tensor=base2.tensor, offset=base2.offset,
                              ap=[base2.ap[0], [4 * CH, 2], [1, ntile * CH]])
            a_sl = a_t[:, t * CH:(t + 1) * CH]
            a_bcast = bass.AP(tensor=a_sl.tensor, offset=a_sl.offset,
                              ap=[a_sl.ap[0], [0, HPC], [1, ntile * CH]])
            nc.vector.tensor_mul(ea_pair, e_pair, a_bcast)

        def new_chunk(ch):
            a_t = apool.tile([128, NJ * CH], bf16, tag="a")
            # sim-time floor keeps later A loads from being queue-ordered
            # ahead of latency-critical small DMAs (e.g. Vt transposes)
            with tc.tile_wait_until(0.016 * ch, enable=ch > 0):
                nc.sync.dma_start(
                    a_t.rearrange("p (t i) -> p t i", i=CH),
                    A3[:, :, ch * CH:(ch + 1) * CH],
                )
            e_t = epool.tile([128, nblocks * CH], bf16, tag="e")
            ea_t = eapool.tile([128, nblocks * CH], bf16, tag="ea")
            return a_t, e_t, ea_t

        # ---- projections all up front (any quarter of Q written mid-stream
        #      forms a false WAR against earlier S-block reads).  Ordered so
        #      the first exp group's deps (K h0, Q quarter 0) come first.
        emit_kproj(0)
        emit_qproj(0)
        emit_kproj(1)
        for quarter in range(1, 4):
            emit_qproj(quarter)
        emit_vproj()

        # ---- main loop.  Per chunk: S-groups with the split exp (ACT) +
        #      Schraudolph tail (DVE stt) inline, EA multiplies emitted as
        #      2-tile ops right after the exp group that completes them (so
        #      no DVE op ahead of a ps-draining stt runs longer than the
        #      exp-paced slot pipeline can absorb), then the lagged phase2
        #      of the previous chunk as one block.
        ngroups = (nblocks + GRP - 1) // GRP
        # EA pair starting at tile st covers st,st+1 -> ready with st+1:
        pair_ready = {st: (blk(0, st + 1) + 4) // GRP
                      for st in range(0, NJ, 2)}

        def emit_chunk_body(ch, e_t, ea_t, a_t):
            for g in range(ngroups):
                emit_sblocks(ch, e_t, g * GRP, min((g + 1) * GRP, nblocks))
                for st, rdy in pair_ready.items():
                    if rdy == g:
                        emit_ea(e_t, ea_t, a_t, st, ntile=2)

        pending = None
        for ch in range(NCH - 1):
            a_t, e_t, ea_t = new_chunk(ch)
            emit_chunk_body(ch, e_t, ea_t, a_t)
            if pending is not None:
                emit_phase2(*pending)
            pending = (e_t, ea_t, ch)

        # ---- last chunk: phase2[2] right after the S blocks, then this
        #      chunk's own phase2 per j-tile pair behind each EA multiply so
        #      the tail after the final exp is ~one tile instead of a chunk.
        ch = NCH - 1
        a_t, e_t, ea_t = new_chunk(ch)
        emit_chunk_body(ch, e_t, ea_t, a_t)
        emit_phase2(*pending)
        od = podpool.tile([128, CH], f32, tag="od")
        for t in range(NJ):
            emit_phase2_tile(od, e_t, ea_t, t)
        o_sb = opool.tile([128, CH], f32, tag="o")
        nc.vector.tensor_copy(o_sb, od)
        nc.sync.dma_start(o[:, ch * CH:(ch + 1) * CH], o_sb)

    nc.finalize()
    return nc


def kernel(h, A, Wq, bq, Wk, bk, Wv, bv):
    global LAST_RESULTS
    from concourse.bass_utils import run_bass_kernel_spmd

    h = np.asarray(h, np.float32)
    A = np.asarray(A, np.float32)
    Wq = np.asarray(Wq, np.float32)
    Wk = np.asarray(Wk, np.float32)
    Wv = np.asarray(Wv, np.float32)
    bq = np.asarray(bq, np.float32)
    bk = np.asarray(bk, np.float32)
    bv = np.asarray(bv, np.float32)

    hT = np.ascontiguousarray(h.transpose(0, 2, 1)).astype(ml_dtypes.bfloat16)
    Ab = np.ascontiguousarray(A.astype(ml_dtypes.bfloat16))  # [B, N, N]
    sc = np.float32(1.0 / math.sqrt(D))

    in_maps = []
    for c in range(NCORES):
        b = c // CORES_PER_B
        h0 = HPC * (c % CORES_PER_B)
        sl = slice(h0 * D, (h0 + HPC) * D)
        bk2 = bk[sl].reshape(HPC, D)                    # [head, d]
        bq2 = (bq[sl] * sc).reshape(HPC, D)
        bcat = np.zeros((128, 5), np.float32)
        for hh in range(HPC):
            bcat[:, 0 + hh] = np.tile(bq2[hh], 4)       # strip-replicated
            bcat[:, 2 + hh] = np.tile(bk2[hh], 4)
        bcat[0:HPC * D, 4] = bv[sl]
        wqkv = np.concatenate([Wq[:, sl], Wk[:, sl], Wv[:, sl]], axis=1)
        in_maps.append({
            "hT": hT[b],
            "Ab": Ab[b],
            "wqkv": np.ascontiguousarray(wqkv).astype(ml_dtypes.bfloat16),
            "bcat": bcat,
        })

    nc = _build_bass()
    res = run_bass_kernel_spmd(
        nc, in_maps, core_ids=list(range(NCORES)),
        trace=os.environ.get("BASS_TRACE", "0") == "1",
    )
    LAST_RESULTS = res

    out = np.empty((B, HEADS, N, D), np.float32)
    for c in range(NCORES):
        b = c // CORES_PER_B
        h0 = HPC * (c % CORES_PER_B)
        oo = res.results[c]["o"]                  # [128, N] f32
        for hh in range(HPC):
            num = oo[hh * D:(hh + 1) * D, :]      # [32, N] unnormalized out^T
            den = oo[64 + 32 * hh, :]             # [N]
            out[b, h0 + hh] = (num / den[None, :]).T
    return out

